# revision 16
# baseline (speedup 1.0000x reference)
"""Trainium2 Bass kernel for nn_CCSequenceModel (2-layer GRU encoder +
autoregressive 2-layer GRU decoder with gated CV head).

Sharding: pure data parallel. B=16384 is split across 8 cores (2048 each).
Per core the recurrent state lives transposed [hidden, batch] and "folded":
batch half A (1024 elems) on partitions 0-63, half B on partitions 64-127,
so every elementwise op runs with all 128 lanes active.

Matmuls: out[M=64 gate dims, N=batch] = lhsT[K, 64].T @ rhs[K, N] with the
tiny GRU weights stationary (bf16, duplicated on partition halves so the
half-B matmuls run in the (64,64) quadrant of the PE array).
"""

import os
import sys

sys.path.insert(0, "/opt/trn_rl_repo")

import numpy as np
import ml_dtypes

import concourse.bass as bass
import concourse.bacc as bacc_mod
import concourse.mybir as mybir
import concourse.tile as tile
from concourse.bass_utils import run_bass_kernel_spmd

F32 = mybir.dt.float32
BF16 = mybir.dt.bfloat16
AF = mybir.ActivationFunctionType
OP = mybir.AluOpType

H = 64
NCORES = 8
BLOC = 2048          # batch per core
NW = 1024            # folded free width (batch half)

# ---- tunables ----
PSUM_DT = F32        # matmul output dtype (bass requires fp32 PSUM)
MM_N = 512           # moving free dim per matmul instruction
BF16_T = ml_dtypes.bfloat16

CELLS = ["e0", "e1", "d0", "d1"]
GPARTS = ["ih_r", "ih_z", "ih_n", "hh_r", "hh_z", "hh_n"]


def _wcols():
    cols = {}
    cur = 0
    for c in CELLS:
        for p in GPARTS:
            cols[c + p] = cur
            cur += 64
    cols["won"] = cur
    cur += 1
    cols["wcv"] = cur
    cur += 1
    return cols, cur


def _bcols():
    cols = {}
    cur = 0
    for c in CELLS:
        for p in ["sr", "sz", "bin", "bhn"]:
            cols[c + p] = cur
            cur += 1
    cols["bon"] = cur
    cur += 1
    cols["bcv"] = cur
    cur += 1
    return cols, cur


WCOLS, NWCOL = _wcols()
BCOLS, NBCOL = _bcols()


def pack_weights(iv):
    """iv: dict of the original reference weights (numpy f32).
    Returns (wpack [128, NWCOL] bf16, bpack [128, NBCOL] f32)."""
    wp = np.zeros((128, NWCOL), np.float32)
    bp = np.zeros((128, NBCOL), np.float32)

    def put_w(col, wt):  # wt [K, 64], duplicated on both partition halves
        k = wt.shape[0]
        wp[0:k, col : col + 64] = wt
        wp[64 : 64 + k, col : col + 64] = wt

    def put_b(col, v):  # v [64] duplicated, or scalar on rows {0, 64}
        v = np.asarray(v, np.float32).reshape(-1)
        bp[0 : v.size, col] = v
        bp[64 : 64 + v.size, col] = v

    cfg = {
        "e0": ("e_Wih0", "e_Whh0", "e_bih0", "e_bhh0"),
        "e1": ("e_Wih1", "e_Whh1", "e_bih1", "e_bhh1"),
        "d0": ("d_Wih0", "d_Whh0", "d_bih0", "d_bhh0"),
        "d1": ("d_Wih1", "d_Whh1", "d_bih1", "d_bhh1"),
    }
    for c, (wih, whh, bih, bhh) in cfg.items():
        Wih, Whh = np.asarray(iv[wih], np.float32), np.asarray(iv[whh], np.float32)
        bih, bhh = np.asarray(iv[bih], np.float32), np.asarray(iv[bhh], np.float32)
        for gi, g in enumerate(["r", "z", "n"]):
            put_w(WCOLS[c + "ih_" + g], Wih[gi * 64 : (gi + 1) * 64, :].T)
            put_w(WCOLS[c + "hh_" + g], Whh[gi * 64 : (gi + 1) * 64, :].T)
        put_b(BCOLS[c + "sr"], bih[0:64] + bhh[0:64])
        put_b(BCOLS[c + "sz"], bih[64:128] + bhh[64:128])
        put_b(BCOLS[c + "bin"], bih[128:192])
        put_b(BCOLS[c + "bhn"], bhh[128:192])
    put_w(WCOLS["won"], np.asarray(iv["W_on"], np.float32).T)  # [64,1]
    put_w(WCOLS["wcv"], np.asarray(iv["W_cv"], np.float32).T)
    put_b(BCOLS["bon"], float(np.asarray(iv["b_on"]).reshape(())))
    put_b(BCOLS["bcv"], float(np.asarray(iv["b_cv"]).reshape(())))
    return wp.astype(BF16_T), bp


def build_program(T, DEC):
    """Emit the per-core Bass program (same program on all 8 cores)."""
    nc = bacc_mod.Bacc(None, target_bir_lowering=False)
    xpack = nc.declare_dram_parameter("xpack", [T, 2, 6, NW], BF16, isOutput=False)
    wpack = nc.declare_dram_parameter("wpack", [128, NWCOL], BF16, isOutput=False)
    bpack = nc.declare_dram_parameter("bpack", [128, NBCOL], F32, isOutput=False)
    outd = nc.declare_dram_parameter("out", [DEC, 2, NW], F32, isOutput=True)

    with tile.TileContext(nc) as tc:
        with (
            tc.tile_pool(name="const", bufs=1) as const,
            tc.tile_pool(name="state", bufs=1) as state,
            tc.tile_pool(name="xin", bufs=4) as xin,
            tc.tile_pool(name="tmp", bufs=3) as tmp,
            tc.tile_pool(name="og", bufs=4) as ogp,
            tc.tile_pool(name="psum", bufs=1, space="PSUM") as psum,
        ):
            wsb = const.tile([128, NWCOL], BF16)
            bsb = const.tile([128, NBCOL], F32)
            nc.gpsimd.dma_start(out=wsb[:, :], in_=wpack[:, :])
            nc.gpsimd.dma_start(out=bsb[:, :], in_=bpack[:, :])

            S0 = state.tile([128, NW], BF16, tag="S0")
            S1 = state.tile([128, NW], BF16, tag="S1")
            PV = state.tile([128, NW], BF16, tag="PV")  # decoder prev on rows {0,64}
            nc.vector.memset(S0[:, :], 0.0)
            nc.vector.memset(S1[:, :], 0.0)
            nc.vector.memset(PV[:, :], 0.0)

            def W(name, k):
                """lhsT AP pair (half A, half B) for weight block `name` with K=k."""
                c = WCOLS[name]
                n = 1 if name in ("won", "wcv") else 64
                return (wsb[0:k, c : c + n], wsb[64 : 64 + k, c : c + n])

            def B_(name):
                c = BCOLS[name]
                return bsb[:, c : c + 1]

            def halves(tile_, k=64):
                return (tile_[0:64, :][0:k, :], tile_[64:128, :][0:k, :])

            # dependency-free matmuls into a dedicated psum bank: keep the PE
            # busy (HAM warm) across the recurrence stalls
            P_d = psum.tile([128, MM_N], PSUM_DT, tag="P_d", bufs=1)
            NDUM = int(os.environ.get("KDUM", "3"))

            def dummies(k):
                for _ in range(k):
                    nc.tensor.matmul(
                        P_d[0:64, :], wsb[0:64, 0:64], wsb[0:64, 0:MM_N],
                        start=True, stop=True, skip_group_check=True,
                    )

            def gate_chunk(P, sl, parts):
                """Accumulate one chunk of a gate pre-act into psum tile P.
                parts: list of ((lhsT_A, lhsT_B), (rhs_A, rhs_B)) contractions."""
                for half in (0, 1):
                    r0 = 0 if half == 0 else 64
                    out = P[r0 : r0 + 64, :]
                    for i, (lt, rh) in enumerate(parts):
                        nc.tensor.matmul(
                            out,
                            lt[half],
                            rh[half][:, sl],
                            start=(i == 0),
                            stop=(i == len(parts) - 1),
                        )

            def gru_cell(cell, in_parts_rz, in_parts_n, hS, outS, ih_first=True):
                """in_parts_rz: per-gate extra input contractions (list for r/z),
                in_parts_n: input contraction for the n 'ih' side (list),
                hS: hidden state tile feeding hh_* parts, outS: state tile updated.

                MM emission is phase-split: the early-available contraction
                (ih when ih_first, else hh) of ALL gates is emitted before any
                late contraction, with accumulation groups interleaved across
                psum banks, so the in-order PE stream never stalls behind a
                single group's late member."""
                hh = halves(hS)
                Z = tmp.tile([128, NW], BF16, tag="Z")
                N_ = tmp.tile([128, NW], BF16, tag="N")

                # allocate psum for both chunks up front; emit ALL early-phase
                # MMs (both chunks) before any late-phase MM so the in-order PE
                # stream has maximal runnable work before the recurrence stall
                chunks = []
                nparts = {}
                seen = {}
                for c0 in range(0, NW, MM_N):
                    sl = slice(c0, c0 + MM_N)
                    P_r = psum.tile([128, MM_N], PSUM_DT, tag="P_r")
                    P_z = psum.tile([128, MM_N], PSUM_DT, tag="P_z")
                    P_i = psum.tile([128, MM_N], PSUM_DT, tag="P_i")
                    P_h = psum.tile([128, MM_N], PSUM_DT, tag="P_h", bufs=1)
                    hhp = {g: [(W(cell + "hh_" + g, 64), hh)] for g in ("r", "z", "n")}
                    if ih_first:
                        phase1 = [(P_r, in_parts_rz("r")), (P_z, in_parts_rz("z")),
                                  (P_i, in_parts_n)]
                        phase2 = [(P_r, hhp["r"]), (P_z, hhp["z"]), (P_h, hhp["n"])]
                    else:
                        phase1 = [(P_r, hhp["r"]), (P_h, hhp["n"]), (P_z, hhp["z"])]
                        phase2 = [(P_r, in_parts_rz("r")), (P_z, in_parts_rz("z")),
                                  (P_i, in_parts_n)]
                    for P, parts in phase1 + phase2:
                        nparts[id(P)] = nparts.get(id(P), 0) + len(parts)
                    chunks.append((sl, P_r, P_z, P_i, P_h, phase1, phase2))

                def emit(P, parts, sl):
                    for lt, rh in parts:
                        i = seen.get(id(P), 0)
                        seen[id(P)] = i + 1
                        for half in (0, 1):
                            r0 = 0 if half == 0 else 64
                            nc.tensor.matmul(
                                P[r0 : r0 + 64, :],
                                lt[half],
                                rh[half][:, sl],
                                start=(i == 0),
                                stop=(i == nparts[id(P)] - 1),
                                skip_group_check=True,
                            )

                for sl, P_r, P_z, P_i, P_h, phase1, phase2 in chunks:
                    for P, parts in phase1:
                        emit(P, parts, sl)
                if not ih_first:
                    dummies(NDUM)
                for sl, P_r, P_z, P_i, P_h, phase1, phase2 in chunks:
                    for P, parts in phase2:
                        emit(P, parts, sl)

                    R = tmp.tile([128, MM_N], BF16, tag="R")
                    A_ = tmp.tile([128, MM_N], BF16, tag="A")
                    Sm = tmp.tile([128, MM_N], BF16, tag="Sm")
                    nc.scalar.activation(R[:, :], P_r[:, :], AF.Sigmoid, bias=B_(cell + "sr"))
                    # A = (hn + bhhn) * r
                    nc.vector.scalar_tensor_tensor(
                        out=A_[:, :], in0=P_h[:, :], scalar=B_(cell + "bhn"),
                        in1=R[:, :], op0=OP.add, op1=OP.mult,
                    )
                    nc.scalar.activation(Z[:, sl], P_z[:, :], AF.Sigmoid, bias=B_(cell + "sz"))
                    # Sm = (inn + bihn) + A
                    nc.vector.scalar_tensor_tensor(
                        out=Sm[:, :], in0=P_i[:, :], scalar=B_(cell + "bin"),
                        in1=A_[:, :], op0=OP.add, op1=OP.add,
                    )
                    nc.scalar.activation(N_[:, sl], Sm[:, :], AF.Tanh)
                    # h' = n + z*(h - n), chunk-granular so the next cell's
                    # matmuls on this chunk can start before the other chunk
                    D = tmp.tile([128, MM_N], BF16, tag="D")
                    E = tmp.tile([128, MM_N], BF16, tag="E")
                    nc.gpsimd.tensor_tensor(out=D[:, :], in0=outS[:, sl], in1=N_[:, sl], op=OP.subtract)
                    nc.vector.tensor_tensor(out=E[:, :], in0=Z[:, sl], in1=D[:, :], op=OP.mult)
                    nc.vector.tensor_tensor(out=outS[:, sl], in0=N_[:, sl], in1=E[:, :], op=OP.add)

            # ---------------- encoder ----------------
            for t in range(T):
                xst = xin.tile([70, NW], BF16, tag="xst")
                nc.gpsimd.dma_start(out=xst[0:6, :], in_=xpack[t, 0])
                nc.gpsimd.dma_start(out=xst[64:70, :], in_=xpack[t, 1])
                xh = (xst[0:6, :], xst[64:70, :])
                gru_cell(
                    "e0",
                    lambda g: [(W("e0ih_" + g, 6), xh)],
                    [(W("e0ih_n", 6), xh)],
                    S0,
                    S0,
                )
                h0h = halves(S0)
                gru_cell(
                    "e1",
                    lambda g: [(W("e1ih_" + g, 64), h0h)],
                    [(W("e1ih_n", 64), h0h)],
                    S1,
                    S1,
                    ih_first=False,
                )

            # ---------------- decoder ----------------
            pvh = (PV[0:1, :], PV[64:65, :])
            rs64 = slice(0, 128, 64)  # rows {0, 64}
            for t in range(DEC):
                gru_cell(
                    "d0",
                    lambda g: [(W("d0ih_" + g, 1), pvh)],
                    [(W("d0ih_n", 1), pvh)],
                    S0,
                    S0,
                    ih_first=False,
                )
                h0h = halves(S0)
                gru_cell(
                    "d1",
                    lambda g: [(W("d1ih_" + g, 64), h0h)],
                    [(W("d1ih_n", 64), h0h)],
                    S1,
                    S1,
                    ih_first=False,
                )
                # heads: logit/cv on psum rows {0,64} (reuse gate psum tags).
                # rows 1..63 compute junk on stale psum, never consumed
                # (partition-strided APs are illegal on engines).
                h1h = halves(S1)
                dummies(NDUM)
                OG = ogp.tile([128, NW], F32, tag="OG")
                r65 = slice(0, 65)
                for c0 in range(0, NW, MM_N):
                    sl = slice(c0, c0 + MM_N)
                    P_on = psum.tile([128, MM_N], PSUM_DT, tag="P_r")
                    P_cv = psum.tile([128, MM_N], PSUM_DT, tag="P_z")
                    for half in (0, 1):
                        r0 = 0 if half == 0 else 64
                        nc.tensor.matmul(
                            P_on[r0 : r0 + 1, :], W("won", 64)[half],
                            h1h[half][:, sl], start=True, stop=True,
                        )
                        nc.tensor.matmul(
                            P_cv[r0 : r0 + 1, :], W("wcv", 64)[half],
                            h1h[half][:, sl], start=True, stop=True,
                        )
                    MK = tmp.tile([128, MM_N], BF16, tag="MK")
                    # mask = (logit + b_on) > 0
                    nc.vector.tensor_scalar(
                        out=MK[r65, :], in0=P_on[r65, :],
                        scalar1=bsb[r65, BCOLS["bon"] : BCOLS["bon"] + 1], scalar2=0.0,
                        op0=OP.add, op1=OP.is_gt,
                    )
                    # gated = (cv + b_cv) * mask
                    nc.vector.scalar_tensor_tensor(
                        out=OG[r65, sl], in0=P_cv[r65, :],
                        scalar=bsb[r65, BCOLS["bcv"] : BCOLS["bcv"] + 1],
                        in1=MK[r65, :], op0=OP.add, op1=OP.mult,
                    )
                nc.gpsimd.tensor_copy(out=PV[r65, :], in_=OG[r65, :])
                nc.sync.dma_start(out=outd[t, 0], in_=OG[0:1, :])
                nc.sync.dma_start(out=outd[t, 1], in_=OG[64:65, :])
    nc.compile()
    return nc


_CACHE = {}


def get_program(T, DEC):
    key = (T, DEC, str(PSUM_DT), MM_N)
    if key not in _CACHE:
        _CACHE[key] = build_program(T, DEC)
    return _CACHE[key]


def pack_x(x):
    """x [B, T, NI] f32 -> per-core list of xpack [T, 2, 6, NW] bf16."""
    B, T, NI = x.shape
    out = []
    for c in range(NCORES):
        xs = x[c * BLOC : (c + 1) * BLOC]  # [2048, T, 6]
        xp = np.ascontiguousarray(
            xs.reshape(2, NW, T, NI).transpose(2, 0, 3, 1)
        )  # [T, 2, 6, NW]
        out.append(xp.astype(BF16_T))
    return out


def run(x, target_len, weights, trace=False, trace_kwargs=None):
    T = x.shape[1]
    DEC = int(target_len)
    nc = get_program(T, DEC)
    wp, bp = pack_weights(weights)
    xps = pack_x(np.asarray(x, np.float32))
    in_maps = [{"xpack": xps[c], "wpack": wp, "bpack": bp} for c in range(NCORES)]
    res = run_bass_kernel_spmd(
        nc, in_maps, list(range(NCORES)), trace=trace, **(trace_kwargs or {})
    )
    outs = [np.asarray(res.results[c]["out"], np.float32) for c in range(NCORES)]
    # [DEC, 2, NW] per core -> [B, DEC, 1]
    full = np.concatenate(
        [o.transpose(1, 2, 0).reshape(BLOC, DEC, 1) for o in outs], axis=0
    )
    return full, res


def kernel(**inputs):
    x = np.asarray(inputs["x"], np.float32)
    target_len = int(np.asarray(inputs["target_len"]).reshape(()))
    weights = {k: v for k, v in inputs.items() if k not in ("x", "target_len")}
    full, _ = run(x, target_len, weights)
    return full.astype(np.float32)


if __name__ == "__main__":
    # tiny smoke test
    rng = np.random.default_rng(0)
    B, T, NI, DEC = 16384, 4, 6, 3
    iv = {
        "x": rng.standard_normal((B, T, NI), dtype=np.float32),
        "target_len": DEC,
    }
    s = 1.0 / np.sqrt(H)
    for nm, shp in [
        ("e_Wih0", (192, 6)), ("e_Whh0", (192, 64)), ("e_bih0", (192,)), ("e_bhh0", (192,)),
        ("e_Wih1", (192, 64)), ("e_Whh1", (192, 64)), ("e_bih1", (192,)), ("e_bhh1", (192,)),
        ("d_Wih0", (192, 1)), ("d_Whh0", (192, 64)), ("d_bih0", (192,)), ("d_bhh0", (192,)),
        ("d_Wih1", (192, 64)), ("d_Whh1", (192, 64)), ("d_bih1", (192,)), ("d_bhh1", (192,)),
        ("W_on", (1, 64)), ("b_on", (1,)), ("W_cv", (1, 64)), ("b_cv", (1,)),
    ]:
        iv[nm] = (rng.uniform(-s, s, shp)).astype(np.float32)
    out = kernel(**iv)
    print("kernel out", out.shape, out.dtype, float(np.abs(out).mean()))


# revision 17
# speedup vs baseline: 1.1602x; 1.1602x over previous
"""Trainium2 Bass kernel for nn_CCSequenceModel (2-layer GRU encoder +
autoregressive 2-layer GRU decoder with gated CV head).

Sharding: pure data parallel. B=16384 is split across 8 cores (2048 each).
Per core the recurrent state lives transposed [hidden, batch] and "folded":
batch half A (1024 elems) on partitions 0-63, half B on partitions 64-127,
so every elementwise op runs with all 128 lanes active.

Matmuls: out[M=64 gate dims, N=batch] = lhsT[K, 64].T @ rhs[K, N] with the
tiny GRU weights stationary (bf16, duplicated on partition halves so the
half-B matmuls run in the (64,64) quadrant of the PE array).
"""

import os
import sys

sys.path.insert(0, "/opt/trn_rl_repo")

import numpy as np
import ml_dtypes

import concourse.bass as bass
import concourse.bacc as bacc_mod
import concourse.mybir as mybir
import concourse.tile as tile
from concourse.bass_utils import run_bass_kernel_spmd

F32 = mybir.dt.float32
BF16 = mybir.dt.bfloat16
AF = mybir.ActivationFunctionType
OP = mybir.AluOpType

H = 64
NCORES = 8
BLOC = 2048          # batch per core
NW = 1024            # folded free width (batch half)

# ---- tunables ----
PSUM_DT = F32        # matmul output dtype (bass requires fp32 PSUM)
MM_N = 512           # moving free dim per matmul instruction
BF16_T = ml_dtypes.bfloat16

CELLS = ["e0", "e1", "d0", "d1"]
GPARTS = ["ih_r", "ih_z", "ih_n", "hh_r", "hh_z", "hh_n"]


def _wcols():
    cols = {}
    cur = 0
    for c in CELLS:
        for p in GPARTS:
            cols[c + p] = cur
            cur += 64
    cols["won"] = cur
    cur += 1
    cols["wcv"] = cur
    cur += 1
    return cols, cur


def _bcols():
    cols = {}
    cur = 0
    for c in CELLS:
        for p in ["sr", "sz", "bin", "bhn"]:
            cols[c + p] = cur
            cur += 1
    cols["bon"] = cur
    cur += 1
    cols["bcv"] = cur
    cur += 1
    return cols, cur


WCOLS, NWCOL = _wcols()
BCOLS, NBCOL = _bcols()


def pack_weights(iv):
    """iv: dict of the original reference weights (numpy f32).
    Returns (wpack [128, NWCOL] bf16, bpack [128, NBCOL] f32)."""
    wp = np.zeros((128, NWCOL), np.float32)
    bp = np.zeros((128, NBCOL), np.float32)

    def put_w(col, wt):  # wt [K, 64], duplicated on both partition halves
        k = wt.shape[0]
        wp[0:k, col : col + 64] = wt
        wp[64 : 64 + k, col : col + 64] = wt

    def put_b(col, v):  # v [64] duplicated, or scalar on rows {0, 64}
        v = np.asarray(v, np.float32).reshape(-1)
        bp[0 : v.size, col] = v
        bp[64 : 64 + v.size, col] = v

    cfg = {
        "e0": ("e_Wih0", "e_Whh0", "e_bih0", "e_bhh0"),
        "e1": ("e_Wih1", "e_Whh1", "e_bih1", "e_bhh1"),
        "d0": ("d_Wih0", "d_Whh0", "d_bih0", "d_bhh0"),
        "d1": ("d_Wih1", "d_Whh1", "d_bih1", "d_bhh1"),
    }
    for c, (wih, whh, bih, bhh) in cfg.items():
        Wih, Whh = np.asarray(iv[wih], np.float32), np.asarray(iv[whh], np.float32)
        bih, bhh = np.asarray(iv[bih], np.float32), np.asarray(iv[bhh], np.float32)
        for gi, g in enumerate(["r", "z", "n"]):
            put_w(WCOLS[c + "ih_" + g], Wih[gi * 64 : (gi + 1) * 64, :].T)
            put_w(WCOLS[c + "hh_" + g], Whh[gi * 64 : (gi + 1) * 64, :].T)
        put_b(BCOLS[c + "sr"], bih[0:64] + bhh[0:64])
        put_b(BCOLS[c + "sz"], bih[64:128] + bhh[64:128])
        put_b(BCOLS[c + "bin"], bih[128:192])
        put_b(BCOLS[c + "bhn"], bhh[128:192])
    put_w(WCOLS["won"], np.asarray(iv["W_on"], np.float32).T)  # [64,1]
    put_w(WCOLS["wcv"], np.asarray(iv["W_cv"], np.float32).T)
    put_b(BCOLS["bon"], float(np.asarray(iv["b_on"]).reshape(())))
    put_b(BCOLS["bcv"], float(np.asarray(iv["b_cv"]).reshape(())))
    return wp.astype(BF16_T), bp


def build_program(T, DEC):
    """Emit the per-core Bass program (same program on all 8 cores)."""
    nc = bacc_mod.Bacc(None, target_bir_lowering=False)
    xpack = nc.declare_dram_parameter("xpack", [T, 2, 6, NW], BF16, isOutput=False)
    wpack = nc.declare_dram_parameter("wpack", [128, NWCOL], BF16, isOutput=False)
    bpack = nc.declare_dram_parameter("bpack", [128, NBCOL], F32, isOutput=False)
    outd = nc.declare_dram_parameter("out", [DEC, 2, NW], F32, isOutput=True)

    with tile.TileContext(nc) as tc:
        with (
            tc.tile_pool(name="const", bufs=1) as const,
            tc.tile_pool(name="state", bufs=1) as state,
            tc.tile_pool(name="xin", bufs=4) as xin,
            tc.tile_pool(name="tmp", bufs=3) as tmp,
            tc.tile_pool(name="og", bufs=4) as ogp,
            tc.tile_pool(name="psum", bufs=1, space="PSUM") as psum,
        ):
            wsb = const.tile([128, NWCOL], BF16)
            bsb = const.tile([128, NBCOL], F32)
            nc.gpsimd.dma_start(out=wsb[:, :], in_=wpack[:, :])
            nc.gpsimd.dma_start(out=bsb[:, :], in_=bpack[:, :])

            S0 = state.tile([128, NW], BF16, tag="S0")
            S1 = state.tile([128, NW], BF16, tag="S1")
            PV = state.tile([128, NW], BF16, tag="PV")  # decoder prev on rows {0,64}
            nc.vector.memset(S0[:, :], 0.0)
            nc.vector.memset(S1[:, :], 0.0)
            nc.vector.memset(PV[:, :], 0.0)

            def W(name, k):
                """lhsT AP pair (half A, half B) for weight block `name` with K=k."""
                c = WCOLS[name]
                n = 1 if name in ("won", "wcv") else 64
                return (wsb[0:k, c : c + n], wsb[64 : 64 + k, c : c + n])

            def B_(name):
                c = BCOLS[name]
                return bsb[:, c : c + 1]

            def halves(tile_, k=64):
                return (tile_[0:64, :][0:k, :], tile_[64:128, :][0:k, :])

            # dependency-free matmuls into a dedicated psum bank: keep the PE
            # busy (HAM warm) across the recurrence stalls
            P_d = psum.tile([128, MM_N], PSUM_DT, tag="P_d", bufs=1)
            NDUM = int(os.environ.get("KDUM", "3"))

            def dummies(k):
                for _ in range(k):
                    nc.tensor.matmul(
                        P_d[0:64, :], wsb[0:64, 0:64], wsb[0:64, 0:MM_N],
                        start=True, stop=True, skip_group_check=True,
                    )

            def gate_chunk(P, sl, parts):
                """Accumulate one chunk of a gate pre-act into psum tile P.
                parts: list of ((lhsT_A, lhsT_B), (rhs_A, rhs_B)) contractions."""
                for half in (0, 1):
                    r0 = 0 if half == 0 else 64
                    out = P[r0 : r0 + 64, :]
                    for i, (lt, rh) in enumerate(parts):
                        nc.tensor.matmul(
                            out,
                            lt[half],
                            rh[half][:, sl],
                            start=(i == 0),
                            stop=(i == len(parts) - 1),
                        )

            def gru_cell(cell, in_parts_rz, in_parts_n, hS, outS, ih_first=True):
                """in_parts_rz: per-gate extra input contractions (list for r/z),
                in_parts_n: input contraction for the n 'ih' side (list),
                hS: hidden state tile feeding hh_* parts, outS: state tile updated.

                MM emission is phase-split: the early-available contraction
                (ih when ih_first, else hh) of ALL gates is emitted before any
                late contraction, with accumulation groups interleaved across
                psum banks, so the in-order PE stream never stalls behind a
                single group's late member."""
                hh = halves(hS)
                Z = tmp.tile([128, NW], BF16, tag="Z")
                N_ = tmp.tile([128, NW], BF16, tag="N")

                # allocate psum for both chunks up front; emit ALL early-phase
                # MMs (both chunks) before any late-phase MM so the in-order PE
                # stream has maximal runnable work before the recurrence stall
                chunks = []
                nparts = {}
                seen = {}
                for c0 in range(0, NW, MM_N):
                    sl = slice(c0, c0 + MM_N)
                    P_r = psum.tile([128, MM_N], PSUM_DT, tag="P_r")
                    P_z = psum.tile([128, MM_N], PSUM_DT, tag="P_z")
                    P_i = psum.tile([128, MM_N], PSUM_DT, tag="P_i")
                    P_h = psum.tile([128, MM_N], PSUM_DT, tag="P_h", bufs=1)
                    hhp = {g: [(W(cell + "hh_" + g, 64), hh)] for g in ("r", "z", "n")}
                    if ih_first:
                        phase1 = [(P_r, in_parts_rz("r")), (P_z, in_parts_rz("z")),
                                  (P_i, in_parts_n)]
                        phase2 = [(P_r, hhp["r"]), (P_z, hhp["z"]), (P_h, hhp["n"])]
                    else:
                        phase1 = [(P_r, hhp["r"]), (P_h, hhp["n"]), (P_z, hhp["z"])]
                        phase2 = [(P_r, in_parts_rz("r")), (P_z, in_parts_rz("z")),
                                  (P_i, in_parts_n)]
                    for P, parts in phase1 + phase2:
                        nparts[id(P)] = nparts.get(id(P), 0) + len(parts)
                    chunks.append((sl, P_r, P_z, P_i, P_h, phase1, phase2))

                def emit(P, parts, sl):
                    for lt, rh in parts:
                        i = seen.get(id(P), 0)
                        seen[id(P)] = i + 1
                        for half in (0, 1):
                            r0 = 0 if half == 0 else 64
                            nc.tensor.matmul(
                                P[r0 : r0 + 64, :],
                                lt[half],
                                rh[half][:, sl],
                                start=(i == 0),
                                stop=(i == nparts[id(P)] - 1),
                                skip_group_check=True,
                            )

                for sl, P_r, P_z, P_i, P_h, phase1, phase2 in chunks:
                    for P, parts in phase1:
                        emit(P, parts, sl)
                if not ih_first:
                    dummies(NDUM)
                for sl, P_r, P_z, P_i, P_h, phase1, phase2 in chunks:
                    for P, parts in phase2:
                        emit(P, parts, sl)

                    R = tmp.tile([128, MM_N], BF16, tag="R")
                    A_ = tmp.tile([128, MM_N], BF16, tag="A")
                    Sm = tmp.tile([128, MM_N], BF16, tag="Sm")
                    nc.scalar.activation(R[:, :], P_r[:, :], AF.Sigmoid, bias=B_(cell + "sr"))
                    # A = (hn + bhhn) * r
                    nc.vector.scalar_tensor_tensor(
                        out=A_[:, :], in0=P_h[:, :], scalar=B_(cell + "bhn"),
                        in1=R[:, :], op0=OP.add, op1=OP.mult,
                    )
                    nc.scalar.activation(Z[:, sl], P_z[:, :], AF.Sigmoid, bias=B_(cell + "sz"))
                    # Sm = (inn + bihn) + A
                    nc.vector.scalar_tensor_tensor(
                        out=Sm[:, :], in0=P_i[:, :], scalar=B_(cell + "bin"),
                        in1=A_[:, :], op0=OP.add, op1=OP.add,
                    )
                    nc.scalar.activation(N_[:, sl], Sm[:, :], AF.Tanh)
                    # h' = n + z*(h - n), chunk-granular so the next cell's
                    # matmuls on this chunk can start before the other chunk
                    D = tmp.tile([128, MM_N], BF16, tag="D")
                    E = tmp.tile([128, MM_N], BF16, tag="E")
                    nc.vector.tensor_tensor(out=D[:, :], in0=outS[:, sl], in1=N_[:, sl], op=OP.subtract)
                    nc.vector.tensor_tensor(out=E[:, :], in0=Z[:, sl], in1=D[:, :], op=OP.mult)
                    nc.vector.tensor_tensor(out=outS[:, sl], in0=N_[:, sl], in1=E[:, :], op=OP.add)

            # ---------------- encoder ----------------
            for t in range(T):
                xst = xin.tile([70, NW], BF16, tag="xst")
                nc.gpsimd.dma_start(out=xst[0:6, :], in_=xpack[t, 0])
                nc.gpsimd.dma_start(out=xst[64:70, :], in_=xpack[t, 1])
                xh = (xst[0:6, :], xst[64:70, :])
                gru_cell(
                    "e0",
                    lambda g: [(W("e0ih_" + g, 6), xh)],
                    [(W("e0ih_n", 6), xh)],
                    S0,
                    S0,
                )
                h0h = halves(S0)
                gru_cell(
                    "e1",
                    lambda g: [(W("e1ih_" + g, 64), h0h)],
                    [(W("e1ih_n", 64), h0h)],
                    S1,
                    S1,
                    ih_first=False,
                )

            # ---------------- decoder ----------------
            pvh = (PV[0:1, :], PV[64:65, :])
            rs64 = slice(0, 128, 64)  # rows {0, 64}
            for t in range(DEC):
                gru_cell(
                    "d0",
                    lambda g: [(W("d0ih_" + g, 1), pvh)],
                    [(W("d0ih_n", 1), pvh)],
                    S0,
                    S0,
                    ih_first=False,
                )
                h0h = halves(S0)
                gru_cell(
                    "d1",
                    lambda g: [(W("d1ih_" + g, 64), h0h)],
                    [(W("d1ih_n", 64), h0h)],
                    S1,
                    S1,
                    ih_first=False,
                )
                # heads: logit/cv on psum rows {0,64} (reuse gate psum tags).
                # rows 1..63 compute junk on stale psum, never consumed
                # (partition-strided APs are illegal on engines).
                h1h = halves(S1)
                dummies(NDUM)
                OG = ogp.tile([128, NW], F32, tag="OG")
                r65 = slice(0, 65)
                for c0 in range(0, NW, MM_N):
                    sl = slice(c0, c0 + MM_N)
                    P_on = psum.tile([128, MM_N], PSUM_DT, tag="P_r")
                    P_cv = psum.tile([128, MM_N], PSUM_DT, tag="P_z")
                    for half in (0, 1):
                        r0 = 0 if half == 0 else 64
                        nc.tensor.matmul(
                            P_on[r0 : r0 + 1, :], W("won", 64)[half],
                            h1h[half][:, sl], start=True, stop=True,
                        )
                        nc.tensor.matmul(
                            P_cv[r0 : r0 + 1, :], W("wcv", 64)[half],
                            h1h[half][:, sl], start=True, stop=True,
                        )
                    MK = tmp.tile([128, MM_N], BF16, tag="MK")
                    # mask = (logit + b_on) > 0
                    nc.vector.tensor_scalar(
                        out=MK[r65, :], in0=P_on[r65, :],
                        scalar1=bsb[r65, BCOLS["bon"] : BCOLS["bon"] + 1], scalar2=0.0,
                        op0=OP.add, op1=OP.is_gt,
                    )
                    # gated = (cv + b_cv) * mask
                    nc.vector.scalar_tensor_tensor(
                        out=OG[r65, sl], in0=P_cv[r65, :],
                        scalar=bsb[r65, BCOLS["bcv"] : BCOLS["bcv"] + 1],
                        in1=MK[r65, :], op0=OP.add, op1=OP.mult,
                    )
                nc.gpsimd.tensor_copy(out=PV[r65, :], in_=OG[r65, :])
                nc.sync.dma_start(out=outd[t, 0], in_=OG[0:1, :])
                nc.sync.dma_start(out=outd[t, 1], in_=OG[64:65, :])
    nc.compile()
    return nc


_CACHE = {}


def get_program(T, DEC):
    key = (T, DEC, str(PSUM_DT), MM_N)
    if key not in _CACHE:
        _CACHE[key] = build_program(T, DEC)
    return _CACHE[key]


def pack_x(x):
    """x [B, T, NI] f32 -> per-core list of xpack [T, 2, 6, NW] bf16."""
    B, T, NI = x.shape
    out = []
    for c in range(NCORES):
        xs = x[c * BLOC : (c + 1) * BLOC]  # [2048, T, 6]
        xp = np.ascontiguousarray(
            xs.reshape(2, NW, T, NI).transpose(2, 0, 3, 1)
        )  # [T, 2, 6, NW]
        out.append(xp.astype(BF16_T))
    return out


def run(x, target_len, weights, trace=False, trace_kwargs=None):
    T = x.shape[1]
    DEC = int(target_len)
    nc = get_program(T, DEC)
    wp, bp = pack_weights(weights)
    xps = pack_x(np.asarray(x, np.float32))
    in_maps = [{"xpack": xps[c], "wpack": wp, "bpack": bp} for c in range(NCORES)]
    res = run_bass_kernel_spmd(
        nc, in_maps, list(range(NCORES)), trace=trace, **(trace_kwargs or {})
    )
    outs = [np.asarray(res.results[c]["out"], np.float32) for c in range(NCORES)]
    # [DEC, 2, NW] per core -> [B, DEC, 1]
    full = np.concatenate(
        [o.transpose(1, 2, 0).reshape(BLOC, DEC, 1) for o in outs], axis=0
    )
    return full, res


def kernel(**inputs):
    x = np.asarray(inputs["x"], np.float32)
    target_len = int(np.asarray(inputs["target_len"]).reshape(()))
    weights = {k: v for k, v in inputs.items() if k not in ("x", "target_len")}
    full, _ = run(x, target_len, weights)
    return full.astype(np.float32)


if __name__ == "__main__":
    # tiny smoke test
    rng = np.random.default_rng(0)
    B, T, NI, DEC = 16384, 4, 6, 3
    iv = {
        "x": rng.standard_normal((B, T, NI), dtype=np.float32),
        "target_len": DEC,
    }
    s = 1.0 / np.sqrt(H)
    for nm, shp in [
        ("e_Wih0", (192, 6)), ("e_Whh0", (192, 64)), ("e_bih0", (192,)), ("e_bhh0", (192,)),
        ("e_Wih1", (192, 64)), ("e_Whh1", (192, 64)), ("e_bih1", (192,)), ("e_bhh1", (192,)),
        ("d_Wih0", (192, 1)), ("d_Whh0", (192, 64)), ("d_bih0", (192,)), ("d_bhh0", (192,)),
        ("d_Wih1", (192, 64)), ("d_Whh1", (192, 64)), ("d_bih1", (192,)), ("d_bhh1", (192,)),
        ("W_on", (1, 64)), ("b_on", (1,)), ("W_cv", (1, 64)), ("b_cv", (1,)),
    ]:
        iv[nm] = (rng.uniform(-s, s, shp)).astype(np.float32)
    out = kernel(**iv)
    print("kernel out", out.shape, out.dtype, float(np.abs(out).mean()))


# revision 18
# speedup vs baseline: 1.3225x; 1.1400x over previous
"""Trainium2 Bass kernel for nn_CCSequenceModel (2-layer GRU encoder +
autoregressive 2-layer GRU decoder with gated CV head).

Sharding: pure data parallel. B=16384 is split across 8 cores (2048 each).
Per core the recurrent state lives transposed [hidden, batch] and "folded":
batch half A (1024 elems) on partitions 0-63, half B on partitions 64-127,
so every elementwise op runs with all 128 lanes active.

Matmuls: out[M=64 gate dims, N=batch] = lhsT[K, 64].T @ rhs[K, N] with the
tiny GRU weights stationary (bf16, duplicated on partition halves so the
half-B matmuls run in the (64,64) quadrant of the PE array).
"""

import os
import sys

sys.path.insert(0, "/opt/trn_rl_repo")

import numpy as np
import ml_dtypes

import concourse.bass as bass
import concourse.bacc as bacc_mod
import concourse.mybir as mybir
import concourse.tile as tile
from concourse.bass_utils import run_bass_kernel_spmd

F32 = mybir.dt.float32
BF16 = mybir.dt.bfloat16
AF = mybir.ActivationFunctionType
OP = mybir.AluOpType

H = 64
NCORES = 8
BLOC = 2048          # batch per core
NW = 1024            # folded free width (batch half)

# ---- tunables ----
PSUM_DT = F32        # matmul output dtype (bass requires fp32 PSUM)
MM_N = 512           # moving free dim per matmul instruction
BF16_T = ml_dtypes.bfloat16

CELLS = ["e0", "e1", "d0", "d1"]
GPARTS = ["ih_r", "ih_z", "ih_n", "hh_r", "hh_z", "hh_n"]


def _wcols():
    cols = {}
    cur = 0
    for c in CELLS:
        for p in GPARTS:
            cols[c + p] = cur
            cur += 64
    cols["won"] = cur
    cur += 1
    cols["wcv"] = cur
    cur += 1
    return cols, cur


def _bcols():
    cols = {}
    cur = 0
    for c in CELLS:
        for p in ["sr", "sz", "bin", "bhn"]:
            cols[c + p] = cur
            cur += 1
    cols["bon"] = cur
    cur += 1
    cols["bcv"] = cur
    cur += 1
    return cols, cur


WCOLS, NWCOL = _wcols()
BCOLS, NBCOL = _bcols()


def pack_weights(iv):
    """iv: dict of the original reference weights (numpy f32).
    Returns (wpack [128, NWCOL] bf16, bpack [128, NBCOL] f32)."""
    wp = np.zeros((128, NWCOL), np.float32)
    bp = np.zeros((128, NBCOL), np.float32)

    def put_w(col, wt):  # wt [K, 64], duplicated on both partition halves
        k = wt.shape[0]
        wp[0:k, col : col + 64] = wt
        wp[64 : 64 + k, col : col + 64] = wt

    def put_b(col, v):  # v [64] duplicated, or scalar on rows {0, 64}
        v = np.asarray(v, np.float32).reshape(-1)
        bp[0 : v.size, col] = v
        bp[64 : 64 + v.size, col] = v

    cfg = {
        "e0": ("e_Wih0", "e_Whh0", "e_bih0", "e_bhh0"),
        "e1": ("e_Wih1", "e_Whh1", "e_bih1", "e_bhh1"),
        "d0": ("d_Wih0", "d_Whh0", "d_bih0", "d_bhh0"),
        "d1": ("d_Wih1", "d_Whh1", "d_bih1", "d_bhh1"),
    }
    for c, (wih, whh, bih, bhh) in cfg.items():
        Wih, Whh = np.asarray(iv[wih], np.float32), np.asarray(iv[whh], np.float32)
        bih, bhh = np.asarray(iv[bih], np.float32), np.asarray(iv[bhh], np.float32)
        for gi, g in enumerate(["r", "z", "n"]):
            put_w(WCOLS[c + "ih_" + g], Wih[gi * 64 : (gi + 1) * 64, :].T)
            put_w(WCOLS[c + "hh_" + g], Whh[gi * 64 : (gi + 1) * 64, :].T)
        put_b(BCOLS[c + "sr"], bih[0:64] + bhh[0:64])
        put_b(BCOLS[c + "sz"], bih[64:128] + bhh[64:128])
        put_b(BCOLS[c + "bin"], bih[128:192])
        put_b(BCOLS[c + "bhn"], bhh[128:192])
    put_w(WCOLS["won"], np.asarray(iv["W_on"], np.float32).T)  # [64,1]
    put_w(WCOLS["wcv"], np.asarray(iv["W_cv"], np.float32).T)
    put_b(BCOLS["bon"], float(np.asarray(iv["b_on"]).reshape(())))
    put_b(BCOLS["bcv"], float(np.asarray(iv["b_cv"]).reshape(())))
    return wp.astype(BF16_T), bp


def build_program(T, DEC):
    """Emit the per-core Bass program (same program on all 8 cores)."""
    nc = bacc_mod.Bacc(None, target_bir_lowering=False)
    xpack = nc.declare_dram_parameter("xpack", [T, 2, 6, NW], BF16, isOutput=False)
    wpack = nc.declare_dram_parameter("wpack", [128, NWCOL], BF16, isOutput=False)
    bpack = nc.declare_dram_parameter("bpack", [128, NBCOL], F32, isOutput=False)
    outd = nc.declare_dram_parameter("out", [DEC, 2, NW], BF16, isOutput=True)

    with tile.TileContext(nc) as tc:
        with (
            tc.tile_pool(name="const", bufs=1) as const,
            tc.tile_pool(name="state", bufs=1) as state,
            tc.tile_pool(name="xin", bufs=4) as xin,
            tc.tile_pool(name="tmp", bufs=3) as tmp,
            tc.tile_pool(name="og", bufs=4) as ogp,
            tc.tile_pool(name="psum", bufs=1, space="PSUM") as psum,
        ):
            wsb = const.tile([128, NWCOL], BF16)
            bsb = const.tile([128, NBCOL], F32)
            nc.gpsimd.dma_start(out=wsb[:, :], in_=wpack[:, :])
            nc.gpsimd.dma_start(out=bsb[:, :], in_=bpack[:, :])

            S0 = state.tile([128, NW], BF16, tag="S0")
            S1 = state.tile([128, NW], BF16, tag="S1")
            PV = state.tile([128, NW], BF16, tag="PV")  # decoder prev on rows {0,64}
            nc.vector.memset(S0[:, :], 0.0)
            nc.vector.memset(S1[:, :], 0.0)
            nc.vector.memset(PV[:, :], 0.0)

            def W(name, k):
                """lhsT AP pair (half A, half B) for weight block `name` with K=k."""
                c = WCOLS[name]
                n = 1 if name in ("won", "wcv") else 64
                return (wsb[0:k, c : c + n], wsb[64 : 64 + k, c : c + n])

            def B_(name):
                c = BCOLS[name]
                return bsb[:, c : c + 1]

            def halves(tile_, k=64):
                return (tile_[0:64, :][0:k, :], tile_[64:128, :][0:k, :])

            # dependency-free matmuls into a dedicated psum bank: keep the PE
            # busy (HAM warm) across the recurrence stalls
            P_d = psum.tile([128, MM_N], PSUM_DT, tag="P_d", bufs=1)
            NDUM = int(os.environ.get("KDUM", "0"))

            def dummies(k):
                for _ in range(k):
                    nc.tensor.matmul(
                        P_d[0:64, :], wsb[0:64, 0:64], wsb[0:64, 0:MM_N],
                        start=True, stop=True, skip_group_check=True,
                    )

            def gate_chunk(P, sl, parts):
                """Accumulate one chunk of a gate pre-act into psum tile P.
                parts: list of ((lhsT_A, lhsT_B), (rhs_A, rhs_B)) contractions."""
                for half in (0, 1):
                    r0 = 0 if half == 0 else 64
                    out = P[r0 : r0 + 64, :]
                    for i, (lt, rh) in enumerate(parts):
                        nc.tensor.matmul(
                            out,
                            lt[half],
                            rh[half][:, sl],
                            start=(i == 0),
                            stop=(i == len(parts) - 1),
                        )

            def gru_cell(cell, in_parts_rz, in_parts_n, hS, outS, ih_first=True):
                """in_parts_rz: per-gate extra input contractions (list for r/z),
                in_parts_n: input contraction for the n 'ih' side (list),
                hS: hidden state tile feeding hh_* parts, outS: state tile updated.

                MM emission is phase-split: the early-available contraction
                (ih when ih_first, else hh) of ALL gates is emitted before any
                late contraction, with accumulation groups interleaved across
                psum banks, so the in-order PE stream never stalls behind a
                single group's late member."""
                hh = halves(hS)
                Z = tmp.tile([128, NW], BF16, tag="Z")
                N_ = tmp.tile([128, NW], BF16, tag="N")

                # allocate psum for both chunks up front; emit ALL early-phase
                # MMs (both chunks) before any late-phase MM so the in-order PE
                # stream has maximal runnable work before the recurrence stall
                chunks = []
                nparts = {}
                seen = {}
                for c0 in range(0, NW, MM_N):
                    sl = slice(c0, c0 + MM_N)
                    P_r = psum.tile([128, MM_N], PSUM_DT, tag="P_r")
                    P_z = psum.tile([128, MM_N], PSUM_DT, tag="P_z")
                    P_i = psum.tile([128, MM_N], PSUM_DT, tag="P_i")
                    P_h = psum.tile([128, MM_N], PSUM_DT, tag="P_h", bufs=1)
                    hhp = {g: [(W(cell + "hh_" + g, 64), hh)] for g in ("r", "z", "n")}
                    if ih_first:
                        phase1 = [(P_r, in_parts_rz("r")), (P_z, in_parts_rz("z")),
                                  (P_i, in_parts_n)]
                        phase2 = [(P_r, hhp["r"]), (P_z, hhp["z"]), (P_h, hhp["n"])]
                    else:
                        phase1 = [(P_r, hhp["r"]), (P_h, hhp["n"]), (P_z, hhp["z"])]
                        phase2 = [(P_r, in_parts_rz("r")), (P_z, in_parts_rz("z")),
                                  (P_i, in_parts_n)]
                    for P, parts in phase1 + phase2:
                        nparts[id(P)] = nparts.get(id(P), 0) + len(parts)
                    chunks.append((sl, P_r, P_z, P_i, P_h, phase1, phase2))

                def emit(P, parts, sl):
                    for lt, rh in parts:
                        i = seen.get(id(P), 0)
                        seen[id(P)] = i + 1
                        for half in (0, 1):
                            r0 = 0 if half == 0 else 64
                            nc.tensor.matmul(
                                P[r0 : r0 + 64, :],
                                lt[half],
                                rh[half][:, sl],
                                start=(i == 0),
                                stop=(i == nparts[id(P)] - 1),
                                skip_group_check=True,
                            )

                for sl, P_r, P_z, P_i, P_h, phase1, phase2 in chunks:
                    for P, parts in phase1:
                        emit(P, parts, sl)
                if not ih_first:
                    dummies(NDUM)
                for sl, P_r, P_z, P_i, P_h, phase1, phase2 in chunks:
                    for P, parts in phase2:
                        emit(P, parts, sl)

                    R = tmp.tile([128, MM_N], BF16, tag="R")
                    A_ = tmp.tile([128, MM_N], BF16, tag="A")
                    Sm = tmp.tile([128, MM_N], BF16, tag="Sm")
                    nc.scalar.activation(R[:, :], P_r[:, :], AF.Sigmoid, bias=B_(cell + "sr"))
                    # A = (hn + bhhn) * r
                    nc.vector.scalar_tensor_tensor(
                        out=A_[:, :], in0=P_h[:, :], scalar=B_(cell + "bhn"),
                        in1=R[:, :], op0=OP.add, op1=OP.mult,
                    )
                    nc.scalar.activation(Z[:, sl], P_z[:, :], AF.Sigmoid, bias=B_(cell + "sz"))
                    # Ib = inn + bihn on ScalarE (has slack; frees a 1x-rate
                    # psum read from the vector engine)
                    Ib = tmp.tile([128, MM_N], BF16, tag="Ib")
                    nc.scalar.activation(Ib[:, :], P_i[:, :], AF.Identity, bias=B_(cell + "bin"))
                    # Sm = Ib + A  (both bf16 SBUF -> 2x mode)
                    nc.vector.tensor_tensor(out=Sm[:, :], in0=Ib[:, :], in1=A_[:, :], op=OP.add)
                    nc.scalar.activation(N_[:, sl], Sm[:, :], AF.Tanh)
                    # h' = n + z*(h - n), chunk-granular so the next cell's
                    # matmuls on this chunk can start before the other chunk
                    D = tmp.tile([128, MM_N], BF16, tag="D")
                    E = tmp.tile([128, MM_N], BF16, tag="E")
                    nc.vector.tensor_tensor(out=D[:, :], in0=outS[:, sl], in1=N_[:, sl], op=OP.subtract)
                    nc.vector.tensor_tensor(out=E[:, :], in0=Z[:, sl], in1=D[:, :], op=OP.mult)
                    nc.vector.tensor_tensor(out=outS[:, sl], in0=N_[:, sl], in1=E[:, :], op=OP.add)

            # ---------------- encoder ----------------
            for t in range(T):
                xst = xin.tile([70, NW], BF16, tag="xst")
                nc.gpsimd.dma_start(out=xst[0:6, :], in_=xpack[t, 0])
                nc.gpsimd.dma_start(out=xst[64:70, :], in_=xpack[t, 1])
                xh = (xst[0:6, :], xst[64:70, :])
                gru_cell(
                    "e0",
                    lambda g: [(W("e0ih_" + g, 6), xh)],
                    [(W("e0ih_n", 6), xh)],
                    S0,
                    S0,
                )
                h0h = halves(S0)
                gru_cell(
                    "e1",
                    lambda g: [(W("e1ih_" + g, 64), h0h)],
                    [(W("e1ih_n", 64), h0h)],
                    S1,
                    S1,
                    ih_first=False,
                )

            # ---------------- decoder ----------------
            pvh = (PV[0:1, :], PV[64:65, :])
            rs64 = slice(0, 128, 64)  # rows {0, 64}
            for t in range(DEC):
                gru_cell(
                    "d0",
                    lambda g: [(W("d0ih_" + g, 1), pvh)],
                    [(W("d0ih_n", 1), pvh)],
                    S0,
                    S0,
                    ih_first=False,
                )
                h0h = halves(S0)
                gru_cell(
                    "d1",
                    lambda g: [(W("d1ih_" + g, 64), h0h)],
                    [(W("d1ih_n", 64), h0h)],
                    S1,
                    S1,
                    ih_first=False,
                )
                # heads: logit/cv on psum rows {0,64} (reuse gate psum tags).
                # rows 1..63 compute junk on stale psum, never consumed
                # (partition-strided APs are illegal on engines).
                h1h = halves(S1)
                dummies(NDUM)
                r65 = slice(0, 65)
                for c0 in range(0, NW, MM_N):
                    sl = slice(c0, c0 + MM_N)
                    P_on = psum.tile([128, MM_N], PSUM_DT, tag="P_r")
                    P_cv = psum.tile([128, MM_N], PSUM_DT, tag="P_z")
                    for half in (0, 1):
                        r0 = 0 if half == 0 else 64
                        nc.tensor.matmul(
                            P_on[r0 : r0 + 1, :], W("won", 64)[half],
                            h1h[half][:, sl], start=True, stop=True,
                        )
                        nc.tensor.matmul(
                            P_cv[r0 : r0 + 1, :], W("wcv", 64)[half],
                            h1h[half][:, sl], start=True, stop=True,
                        )
                    MK = tmp.tile([128, MM_N], BF16, tag="MK")
                    # mask = (logit + b_on) > 0
                    nc.vector.tensor_scalar(
                        out=MK[r65, :], in0=P_on[r65, :],
                        scalar1=bsb[r65, BCOLS["bon"] : BCOLS["bon"] + 1], scalar2=0.0,
                        op0=OP.add, op1=OP.is_gt,
                    )
                    # gated = (cv + b_cv) * mask, written bf16 straight into
                    # the recurrence input PV (host upconverts the output)
                    nc.vector.scalar_tensor_tensor(
                        out=PV[r65, sl], in0=P_cv[r65, :],
                        scalar=bsb[r65, BCOLS["bcv"] : BCOLS["bcv"] + 1],
                        in1=MK[r65, :], op0=OP.add, op1=OP.mult,
                    )
                nc.sync.dma_start(out=outd[t, 0], in_=PV[0:1, :])
                nc.sync.dma_start(out=outd[t, 1], in_=PV[64:65, :])
    nc.compile()
    return nc


_CACHE = {}


def get_program(T, DEC):
    key = (T, DEC, str(PSUM_DT), MM_N)
    if key not in _CACHE:
        _CACHE[key] = build_program(T, DEC)
    return _CACHE[key]


def pack_x(x):
    """x [B, T, NI] f32 -> per-core list of xpack [T, 2, 6, NW] bf16."""
    B, T, NI = x.shape
    out = []
    for c in range(NCORES):
        xs = x[c * BLOC : (c + 1) * BLOC]  # [2048, T, 6]
        xp = np.ascontiguousarray(
            xs.reshape(2, NW, T, NI).transpose(2, 0, 3, 1)
        )  # [T, 2, 6, NW]
        out.append(xp.astype(BF16_T))
    return out


def run(x, target_len, weights, trace=False, trace_kwargs=None):
    T = x.shape[1]
    DEC = int(target_len)
    nc = get_program(T, DEC)
    wp, bp = pack_weights(weights)
    xps = pack_x(np.asarray(x, np.float32))
    in_maps = [{"xpack": xps[c], "wpack": wp, "bpack": bp} for c in range(NCORES)]
    res = run_bass_kernel_spmd(
        nc, in_maps, list(range(NCORES)), trace=trace, **(trace_kwargs or {})
    )
    outs = [np.asarray(res.results[c]["out"], np.float32) for c in range(NCORES)]
    # [DEC, 2, NW] per core -> [B, DEC, 1]
    full = np.concatenate(
        [o.transpose(1, 2, 0).reshape(BLOC, DEC, 1) for o in outs], axis=0
    )
    return full, res


def kernel(**inputs):
    x = np.asarray(inputs["x"], np.float32)
    target_len = int(np.asarray(inputs["target_len"]).reshape(()))
    weights = {k: v for k, v in inputs.items() if k not in ("x", "target_len")}
    full, _ = run(x, target_len, weights)
    return full.astype(np.float32)


if __name__ == "__main__":
    # tiny smoke test
    rng = np.random.default_rng(0)
    B, T, NI, DEC = 16384, 4, 6, 3
    iv = {
        "x": rng.standard_normal((B, T, NI), dtype=np.float32),
        "target_len": DEC,
    }
    s = 1.0 / np.sqrt(H)
    for nm, shp in [
        ("e_Wih0", (192, 6)), ("e_Whh0", (192, 64)), ("e_bih0", (192,)), ("e_bhh0", (192,)),
        ("e_Wih1", (192, 64)), ("e_Whh1", (192, 64)), ("e_bih1", (192,)), ("e_bhh1", (192,)),
        ("d_Wih0", (192, 1)), ("d_Whh0", (192, 64)), ("d_bih0", (192,)), ("d_bhh0", (192,)),
        ("d_Wih1", (192, 64)), ("d_Whh1", (192, 64)), ("d_bih1", (192,)), ("d_bhh1", (192,)),
        ("W_on", (1, 64)), ("b_on", (1,)), ("W_cv", (1, 64)), ("b_cv", (1,)),
    ]:
        iv[nm] = (rng.uniform(-s, s, shp)).astype(np.float32)
    out = kernel(**iv)
    print("kernel out", out.shape, out.dtype, float(np.abs(out).mean()))
